# revision 25
# baseline (speedup 1.0000x reference)
"""Bass/Tile TRN2 kernel for nn_CrossEncoder (RoPE cross-attention, returns (ctx, attn)).

Sharding: 8 cores = (batch b, q-half). Core c handles batch c//2, query rows
[(c%2)*512, (c%2)*512+512). No collectives; each core computes its output slice:
  ctx slice  [512, 512]         -> ctx[b, qo:qo+512, :]
  attn slice [8, 512, 2048]     -> attn[b, :, qo:qo+512, :]

Matmul operands use float32r (fp32 bits, PE rounds to ~12-bit mantissa, runs at
1 cycle/row vs fp32's LOW_HIGH 2-pass) — measured 1.6e-4 matmul rel err.

Per-core pipeline:
  - projections computed transposed: qhT/khT [d-on-partition, t-on-free];
    vh in normal [k, d] layout with a ones column appended per head so the
    ctx matmul emits the softmax denominator row for free.
  - RoPE: y = x*cos + (Mrot @ x)*sin; cos/sin tables precomputed host-side.
  - scores [q,k]: K=64 matmuls + a K=1 rank-1 matmul adding -1e9 at masked k.
  - softmax: ACT exp(0.125*x) with accum_out row sums; GPSIMD normalize;
    contiguous 1MB attn stores.
  - ctx: scores^T [k,q] (mask via per-partition ACT bias) -> exp -> f32r;
    ctxT[d(+Z row), q] = vh_aug.T @ pT accumulated over k; scaled by the
    broadcast reciprocal of its own Z row; out-proj per head with K=64.
"""

import os
import sys
from contextlib import ExitStack

import numpy as np

sys.path.insert(0, "/opt/trn_rl_repo")

import concourse.bass as bass
import concourse.tile as tile
from concourse import bacc, mybir
from concourse.bass_utils import run_bass_kernel_spmd

B, QT, N, E, H, Dh = 4, 1024, 2048, 512, 8, 64
QS = 512          # q rows per core
P = 128
NC = 8
ROPE_BASE = 10000.0
SCALE = 1.0 / 8.0  # 1/sqrt(Dh)
F32 = mybir.dt.float32
F32R = mybir.dt.float32r
BF16 = mybir.dt.bfloat16

_NC_CACHE = {}


def _rot_matrix_T():
    """M s.t. (M @ x)[2p] = -x[2p+1], (M @ x)[2p+1] = x[2p]; returns M.T for lhsT."""
    M = np.zeros((P, P), np.float32)
    for p in range(P // 2):
        M[2 * p, 2 * p + 1] = -1.0
        M[2 * p + 1, 2 * p] = 1.0
    return np.ascontiguousarray(M.T)


def build_nc():
    nc = bacc.Bacc(None, target_bir_lowering=False)

    # ---- per-core external inputs (f32r = raw fp32 bits, PE rounds) ----
    qT_d = nc.dram_tensor("qT", [4, P, QS], F32R, kind="ExternalInput")
    kvT_d = nc.dram_tensor("kvT", [4, P, N], F32R, kind="ExternalInput")
    Wq_d = nc.dram_tensor("Wq", [4, P, E], F32R, kind="ExternalInput")
    Wk_d = nc.dram_tensor("Wk", [4, P, E], F32R, kind="ExternalInput")
    Wv_d = nc.dram_tensor("Wv", [4, P, E], F32R, kind="ExternalInput")
    Wo_d = nc.dram_tensor("Wo", [H, Dh, E], F32R, kind="ExternalInput")
    bq_d = nc.dram_tensor("bq", [P, 4], F32, kind="ExternalInput")
    bk_d = nc.dram_tensor("bk", [P, 4], F32, kind="ExternalInput")
    bv_d = nc.dram_tensor("bv", [1, E], F32R, kind="ExternalInput")
    bo_d = nc.dram_tensor("bo", [1, E], F32R, kind="ExternalInput")
    cosq_d = nc.dram_tensor("cosq", [P, QS], F32, kind="ExternalInput")
    sinq_d = nc.dram_tensor("sinq", [P, QS], F32, kind="ExternalInput")
    coskv_d = nc.dram_tensor("coskv", [P, N], F32, kind="ExternalInput")
    sinkv_d = nc.dram_tensor("sinkv", [P, N], F32, kind="ExternalInput")
    maskneg_d = nc.dram_tensor("maskneg", [1, N], F32R, kind="ExternalInput")
    maskbias_d = nc.dram_tensor("maskbias", [P, 16], F32, kind="ExternalInput")
    qmask_d = nc.dram_tensor("qmask", [P, 4], F32, kind="ExternalInput")

    # constants baked into the NEFF (same on every core)
    rotMT_d = nc.inline_tensor(_rot_matrix_T(), name="rotMT")
    ones_d = nc.inline_tensor(np.ones((P, P), np.float32), name="onesc")

    # ---- outputs ----
    attn_o = nc.dram_tensor("attn_o", [H, QS, N], F32, kind="ExternalOutput")
    ctx_o = nc.dram_tensor("ctx_o", [QS, E], F32, kind="ExternalOutput")

    with tile.TileContext(nc) as tc, ExitStack() as ctx:
        persist = ctx.enter_context(tc.tile_pool(name="persist", bufs=1))
        khT = persist.tile([P, 4, N], F32R)       # pair p: rows = dims of heads 2p,2p+1
        qhT = persist.tile([P, 4, QS], F32R)
        vh_aug = persist.tile([P, 16, 8, 65], F32R)  # [k-chunk][128 k][head][64 d + 1]
        ctxT_sb = persist.tile([64, H, QS], F32R)    # normalized ctx^T per head
        zrec = persist.tile([P, H, 4], F32)          # 1/Z per (head, q-chunk), q on part
        rotMT = persist.tile([P, P], F32R)
        ones_r = persist.tile([1, P], F32R)
        bq_sb = persist.tile([P, 4], F32)
        bk_sb = persist.tile([P, 4], F32)
        bv_sb = persist.tile([1, E], F32R)
        bo_sb = persist.tile([1, E], F32R)
        maskbias = persist.tile([P, 16], F32)
        qmask_sb = persist.tile([P, 4], F32)

        nc.sync.dma_start(out=rotMT, in_=rotMT_d[:].bitcast(F32R))
        nc.sync.dma_start(out=ones_r, in_=ones_d[0:1, :].bitcast(F32R))
        nc.sync.dma_start(out=bq_sb, in_=bq_d[:])
        nc.sync.dma_start(out=bk_sb, in_=bk_d[:])
        nc.sync.dma_start(out=bv_sb, in_=bv_d[:])
        nc.sync.dma_start(out=bo_sb, in_=bo_d[:])
        nc.sync.dma_start(out=maskbias, in_=maskbias_d[:])
        nc.sync.dma_start(out=qmask_sb, in_=qmask_d[:])
        # ones columns of vh_aug (col 64 of each head block), all k chunks
        ones_cols = bass.AP(
            tensor=ones_d.ap().tensor,
            offset=0,
            ap=[[P, P], [0, 128]],
        ).bitcast(F32R)
        nc.sync.dma_start(out=vh_aug[:, :, :, 64], in_=ones_cols)

        # ================= phase 0: projections + RoPE =================
        with (
            tc.tile_pool(name="ph0", bufs=2) as ph0,
            tc.tile_pool(name="ph0c", bufs=1) as ph0c,
        ):
            kvT_sb = ph0c.tile([P, 4, N], F32R)
            qT_sb = ph0c.tile([P, 4, QS], F32R)
            Wq_sb = ph0c.tile([P, 4, E], F32R)
            Wk_sb = ph0c.tile([P, 4, E], F32R)
            Wv_sb = ph0c.tile([P, 4, E], F32R)
            cosq = ph0c.tile([P, QS], F32)
            sinq = ph0c.tile([P, QS], F32)
            coskv = ph0c.tile([P, N], F32)
            sinkv = ph0c.tile([P, N], F32)
            nc.sync.dma_start(out=cosq, in_=cosq_d[:])
            nc.sync.dma_start(out=sinq, in_=sinq_d[:])
            nc.sync.dma_start(out=coskv, in_=coskv_d[:])
            nc.sync.dma_start(out=sinkv, in_=sinkv_d[:])
            for e in range(4):
                nc.sync.dma_start(out=kvT_sb[:, e, :], in_=kvT_d[e])
                nc.sync.dma_start(out=qT_sb[:, e, :], in_=qT_d[e])
                nc.sync.dma_start(out=Wq_sb[:, e, :], in_=Wq_d[e])
                nc.sync.dma_start(out=Wk_sb[:, e, :], in_=Wk_d[e])
                nc.sync.dma_start(out=Wv_sb[:, e, :], in_=Wv_d[e])

            # V projection: vh[kc] = kv[kc] @ Wv + bv   ([k, d] layout + Z col)
            with tc.tile_pool(name="psV", bufs=3, space="PSUM") as psV:
                for kc in range(16):
                    psv = psV.tile([P, E], F32, tag="psv")
                    for e in range(4):
                        nc.tensor.matmul(
                            psv,
                            kvT_sb[:, e, kc * P:(kc + 1) * P],
                            Wv_sb[:, e, :],
                            start=(e == 0),
                            stop=False,
                        )
                    nc.tensor.matmul(psv, ones_r, bv_sb, start=False, stop=True)
                    nc.vector.tensor_copy(
                        vh_aug[:, kc, :, 0:64],
                        psv.rearrange("p (h d) -> p h d", h=8),
                    )

            # Q/K projections (transposed layout) + RoPE, per head pair
            with tc.tile_pool(name="ph0ps", bufs=2, space="PSUM") as ph0ps:
                for hp in range(4):
                    psq = ph0ps.tile([P, QS], F32, tag="psq")
                    for e in range(4):
                        nc.tensor.matmul(
                            psq,
                            Wq_sb[:, e, hp * P:(hp + 1) * P],
                            qT_sb[:, e, :],
                            start=(e == 0),
                            stop=(e == 3),
                        )
                    qraw = ph0.tile([P, QS], F32R, tag="qraw")
                    nc.vector.tensor_scalar_add(qraw, psq, bq_sb[:, hp:hp + 1])
                    psrot = ph0ps.tile([P, QS], F32, tag="psrot")
                    nc.tensor.matmul(psrot, rotMT, qraw, start=True, stop=True)
                    t1 = ph0.tile([P, QS], F32, tag="t1")
                    nc.vector.tensor_mul(t1, qraw, cosq)
                    t2 = ph0.tile([P, QS], F32, tag="t2")
                    nc.vector.tensor_mul(t2, psrot, sinq)
                    nc.vector.tensor_add(qhT[:, hp, :], t1, t2)

                    for c4 in range(4):
                        psk = ph0ps.tile([P, 512], F32, tag="psk")
                        for e in range(4):
                            nc.tensor.matmul(
                                psk,
                                Wk_sb[:, e, hp * P:(hp + 1) * P],
                                kvT_sb[:, e, c4 * 512:(c4 + 1) * 512],
                                start=(e == 0),
                                stop=(e == 3),
                            )
                        kraw = ph0.tile([P, 512], F32R, tag="kraw")
                        nc.vector.tensor_scalar_add(kraw, psk, bk_sb[:, hp:hp + 1])
                        psrotk = ph0ps.tile([P, 512], F32, tag="psrotk")
                        nc.tensor.matmul(psrotk, rotMT, kraw, start=True, stop=True)
                        t1k = ph0.tile([P, 512], F32, tag="t1k")
                        nc.vector.tensor_mul(
                            t1k, kraw, coskv[:, c4 * 512:(c4 + 1) * 512])
                        t2k = ph0.tile([P, 512], F32, tag="t2k")
                        nc.vector.tensor_mul(
                            t2k, psrotk, sinkv[:, c4 * 512:(c4 + 1) * 512])
                        nc.vector.tensor_add(
                            khT[:, hp, c4 * 512:(c4 + 1) * 512], t1k, t2k)

        # ================= main loop: per head =================
        with (
            tc.tile_pool(name="mn", bufs=2) as mn,
            tc.tile_pool(name="mn1", bufs=1) as mn1,
            tc.tile_pool(name="mnsm", bufs=4) as mnsm,
            tc.tile_pool(name="psA", bufs=2, space="PSUM") as psA,
            tc.tile_pool(name="psB", bufs=2, space="PSUM") as psB,
            tc.tile_pool(name="psC", bufs=2, space="PSUM") as psC,
        ):
            maskneg = mn1.tile([1, N], F32R, tag="maskneg")
            nc.sync.dma_start(out=maskneg, in_=maskneg_d[:])
            for h in range(H):
                hp, hs = h // 2, h % 2
                dsl = slice(hs * 64, hs * 64 + 64)

                # ---- phase A: scores [q, k], softmax, attn output ----
                for qc in range(4):
                    ptile = mn.tile([P, N], F32, tag="ptile")
                    zparts = mnsm.tile([P, 2], F32, tag="zparts")
                    for kh in range(2):
                        ps = psA.tile([P, 2, 512], F32, tag="ps_scores")
                        for kq in range(2):
                            kc4 = kh * 2 + kq
                            nc.tensor.matmul(
                                ps[:, kq, :],
                                qhT[dsl, hp, qc * P:(qc + 1) * P],
                                khT[dsl, hp, kc4 * 512:(kc4 + 1) * 512],
                                start=True,
                                stop=False,
                            )
                            nc.tensor.matmul(
                                ps[:, kq, :],
                                ones_r,
                                maskneg[:, kc4 * 512:(kc4 + 1) * 512],
                                start=False,
                                stop=True,
                            )
                        nc.scalar.activation(
                            out=ptile[:, kh * 1024:(kh + 1) * 1024],
                            in_=ps.rearrange("p a b -> p (a b)"),
                            func=mybir.ActivationFunctionType.Exp,
                            scale=SCALE,
                            accum_out=zparts[:, kh:kh + 1],
                        )
                    z1 = mnsm.tile([P, 1], F32, tag="z1")
                    nc.vector.reduce_sum(out=z1, in_=zparts, axis=mybir.AxisListType.X)
                    nc.vector.reciprocal(out=zrec[:, h, qc:qc + 1], in_=z1)
                    attn_t = mn.tile([P, N], F32, tag="attn_t")
                    nc.vector.tensor_scalar_mul(attn_t, ptile, zrec[:, h, qc:qc + 1])
                    nc.sync.dma_start(
                        out=attn_o[h, qc * P:(qc + 1) * P, :], in_=attn_t
                    )

                # ---- phase B: scores^T [k, q] -> exp(f32r) tiles for all k ----
                pT_all = mn1.tile([P, 16, 512], F32R, tag="pT_all")
                for kc in range(16):
                    psT = psB.tile([P, 512], F32, tag="psT")
                    nc.tensor.matmul(
                        psT,
                        khT[dsl, hp, kc * P:(kc + 1) * P],
                        qhT[dsl, hp, :],
                        start=True,
                        stop=True,
                    )
                    nc.scalar.activation(
                        out=pT_all[:, kc, :],
                        in_=psT,
                        func=mybir.ActivationFunctionType.Exp,
                        scale=SCALE,
                        bias=maskbias[:, kc:kc + 1],
                    )
                # ---- phase C: ctxT (+Z row) accumulation, scale by 1/Z ----
                ctxZ = psC.tile([65, QS], F32, tag="ctxZ")
                for kc in range(16):
                    nc.tensor.matmul(
                        ctxZ,
                        vh_aug[:, kc, h, :],
                        pT_all[:, kc, :],
                        start=(kc == 0),
                        stop=(kc == 15),
                    )
                zrow = mnsm.tile([1, QS], F32, tag="zrow")
                nc.vector.reciprocal(out=zrow, in_=ctxZ[64:65, :])
                zbc = mnsm.tile([64, QS], F32, tag="zbc")
                nc.gpsimd.partition_broadcast(zbc, zrow)
                nc.vector.tensor_mul(ctxT_sb[:, h, :], ctxZ[0:64, :], zbc)

        # ================= output projection =================
        with (
            tc.tile_pool(name="fin", bufs=2) as fin,
            tc.tile_pool(name="psF", bufs=2, space="PSUM") as psF,
        ):
            Wo_sb = fin.tile([64, H, E], F32R, tag="Wo_sb")
            for hh in range(H):
                nc.sync.dma_start(out=Wo_sb[:, hh, :], in_=Wo_d[hh])
            for qc in range(4):
                pso = psF.tile([P, E], F32, tag="pso")
                for hh in range(H):
                    nc.tensor.matmul(
                        pso,
                        ctxT_sb[:, hh, qc * P:(qc + 1) * P],
                        Wo_sb[:, hh, :],
                        start=(hh == 0),
                        stop=False,
                    )
                nc.tensor.matmul(pso, ones_r, bo_sb, start=False, stop=True)
                octx = fin.tile([P, E], F32, tag="octx")
                nc.scalar.activation(
                    out=octx, in_=pso,
                    func=mybir.ActivationFunctionType.Copy,
                    scale=qmask_sb[:, qc:qc + 1],
                )
                nc.sync.dma_start(out=ctx_o[qc * P:(qc + 1) * P, :], in_=octx)

    nc.finalize()
    return nc


def _cos_sin_tiles(pos_idx):
    """Replicates reference rope_cos_sin_from_pos + the rope_apply slicing quirk.

    Returns (cos_tile, sin_tile) each [128, T] f32: row d holds the factor for
    rotated dim d of one head (pattern repeated for the second head in a pair).
    """
    half = Dh // 2
    freqs = (1.0 / (ROPE_BASE ** (np.arange(half, dtype=np.float32) / np.float32(half)))).astype(np.float32)
    ang = pos_idx.astype(np.float32)[:, None] * freqs[None, :]       # [T, 32]
    cos_full = np.repeat(np.cos(ang).astype(np.float32), 2, axis=1)  # [T, 64]
    sin_full = np.repeat(np.sin(ang).astype(np.float32), 2, axis=1)
    c = cos_full[:, :half]                                           # [T, 32] (quirk)
    s = sin_full[:, :half]
    C = np.repeat(c, 2, axis=1).T                                    # [64, T]
    S = np.repeat(s, 2, axis=1).T
    return (
        np.ascontiguousarray(np.vstack([C, C]), dtype=np.float32),
        np.ascontiguousarray(np.vstack([S, S]), dtype=np.float32),
    )


def kernel(q, kv, q_mask, kv_mask, q_pos_idx, kv_pos_idx,
           Wq, bq, Wk, bk, Wv, bv, Wo, bo):
    q = np.asarray(q, np.float32)
    kv = np.asarray(kv, np.float32)
    q_mask = np.asarray(q_mask)
    kv_mask = np.asarray(kv_mask)
    q_pos_idx = np.asarray(q_pos_idx)
    kv_pos_idx = np.asarray(kv_pos_idx)
    Wq, Wk, Wv, Wo = (np.asarray(x, np.float32) for x in (Wq, Wk, Wv, Wo))
    bq, bk, bv, bo = (np.asarray(x, np.float32) for x in (bq, bk, bv, bo))

    if "nc" not in _NC_CACHE:
        _NC_CACHE["nc"] = build_nc()
    nc = _NC_CACHE["nc"]

    w_maps = {
        "Wq": np.ascontiguousarray(Wq.reshape(4, P, E)),
        "Wk": np.ascontiguousarray(Wk.reshape(4, P, E)),
        "Wv": np.ascontiguousarray(Wv.reshape(4, P, E)),
        "Wo": np.ascontiguousarray(Wo.reshape(H, Dh, E)),
        "bq": np.ascontiguousarray(bq.reshape(4, P).T),
        "bk": np.ascontiguousarray(bk.reshape(4, P).T),
        "bv": np.ascontiguousarray(bv[None, :]),
        "bo": np.ascontiguousarray(bo[None, :]),
    }

    in_maps = []
    for c in range(NC):
        b, qo = c // 2, (c % 2) * QS
        coskv_t, sinkv_t = _cos_sin_tiles(kv_pos_idx[b])
        cosq_t, sinq_t = _cos_sin_tiles(q_pos_idx[b, qo:qo + QS])
        mneg = np.where(kv_mask[b], 0.0, -1e9).astype(np.float32)
        m = {
            "qT": np.ascontiguousarray(q[b, qo:qo + QS, :].T.reshape(4, P, QS)),
            "kvT": np.ascontiguousarray(kv[b].T.reshape(4, P, N)),
            "cosq": cosq_t, "sinq": sinq_t,
            "coskv": coskv_t, "sinkv": sinkv_t,
            "maskneg": np.ascontiguousarray(mneg[None, :]),
            "maskbias": np.ascontiguousarray(mneg.reshape(16, P).T),
            "qmask": np.ascontiguousarray(
                q_mask[b, qo:qo + QS].astype(np.float32).reshape(4, P).T),
        }
        m.update(w_maps)
        in_maps.append(m)

    res = run_bass_kernel_spmd(nc, in_maps, core_ids=list(range(NC)))

    ctx = np.empty((B, QT, E), np.float32)
    attn = np.empty((B, H, QT, N), np.float32)
    for c in range(NC):
        b, qo = c // 2, (c % 2) * QS
        ctx[b, qo:qo + QS, :] = res.results[c]["ctx_o"]
        attn[b, :, qo:qo + QS, :] = res.results[c]["attn_o"]
    return ctx, attn


# revision 31
# speedup vs baseline: 1.0031x; 1.0031x over previous
"""Bass/Tile TRN2 kernel for nn_CrossEncoder (RoPE cross-attention, returns (ctx, attn)).

Sharding: 8 cores = (batch b, q-half). Core c handles batch c//2, query rows
[(c%2)*512, (c%2)*512+512). No collectives; each core computes its output slice:
  ctx slice  [512, 512]         -> ctx[b, qo:qo+512, :]
  attn slice [8, 512, 2048]     -> attn[b, :, qo:qo+512, :]

Matmul operands use float32r (fp32 bits, PE rounds to ~12-bit mantissa, runs at
1 cycle/row vs fp32's LOW_HIGH 2-pass) — measured 1.6e-4 matmul rel err.

Per-core pipeline:
  - projections computed transposed: qhT/khT [d-on-partition, t-on-free];
    vh in normal [k, d] layout with a ones column appended per head so the
    ctx matmul emits the softmax denominator row for free.
  - RoPE: y = x*cos + (Mrot @ x)*sin; cos/sin tables precomputed host-side.
  - scores [q,k]: K=64 matmuls + a K=1 rank-1 matmul adding -1e9 at masked k.
  - softmax: ACT exp(0.125*x) with accum_out row sums; GPSIMD normalize;
    contiguous 1MB attn stores.
  - ctx: scores^T [k,q] (mask via per-partition ACT bias) -> exp -> f32r;
    ctxT[d(+Z row), q] = vh_aug.T @ pT accumulated over k; scaled by the
    broadcast reciprocal of its own Z row; out-proj per head with K=64.
"""

import os
import sys
from contextlib import ExitStack

import numpy as np

sys.path.insert(0, "/opt/trn_rl_repo")

import concourse.bass as bass
import concourse.tile as tile
from concourse import bacc, mybir
from concourse.bass_utils import run_bass_kernel_spmd

B, QT, N, E, H, Dh = 4, 1024, 2048, 512, 8, 64
QS = 512          # q rows per core
P = 128
NC = 8
ROPE_BASE = 10000.0
SCALE = 1.0 / 8.0  # 1/sqrt(Dh)
F32 = mybir.dt.float32
F32R = mybir.dt.float32r
BF16 = mybir.dt.bfloat16

_NC_CACHE = {}


def _rot_matrix_T():
    """M s.t. (M @ x)[2p] = -x[2p+1], (M @ x)[2p+1] = x[2p]; returns M.T for lhsT."""
    M = np.zeros((P, P), np.float32)
    for p in range(P // 2):
        M[2 * p, 2 * p + 1] = -1.0
        M[2 * p + 1, 2 * p] = 1.0
    return np.ascontiguousarray(M.T)


def build_nc():
    nc = bacc.Bacc(None, target_bir_lowering=False)

    # ---- per-core external inputs (f32r = raw fp32 bits, PE rounds) ----
    qT_d = nc.dram_tensor("qT", [4, P, QS], F32R, kind="ExternalInput")
    kvT_d = nc.dram_tensor("kvT", [4, P, N], F32R, kind="ExternalInput")
    Wq_d = nc.dram_tensor("Wq", [4, P, E], F32R, kind="ExternalInput")
    Wk_d = nc.dram_tensor("Wk", [4, P, E], F32R, kind="ExternalInput")
    Wv_d = nc.dram_tensor("Wv", [4, P, E], F32R, kind="ExternalInput")
    Wo_d = nc.dram_tensor("Wo", [H, Dh, E], F32R, kind="ExternalInput")
    bq_d = nc.dram_tensor("bq", [P, 4], F32, kind="ExternalInput")
    bk_d = nc.dram_tensor("bk", [P, 4], F32, kind="ExternalInput")
    bv_d = nc.dram_tensor("bv", [1, E], F32R, kind="ExternalInput")
    bo_d = nc.dram_tensor("bo", [1, E], F32R, kind="ExternalInput")
    cosq_d = nc.dram_tensor("cosq", [P, QS], F32, kind="ExternalInput")
    sinq_d = nc.dram_tensor("sinq", [P, QS], F32, kind="ExternalInput")
    coskv_d = nc.dram_tensor("coskv", [P, N], F32, kind="ExternalInput")
    sinkv_d = nc.dram_tensor("sinkv", [P, N], F32, kind="ExternalInput")
    maskneg_d = nc.dram_tensor("maskneg", [1, N], F32R, kind="ExternalInput")
    maskbias_d = nc.dram_tensor("maskbias", [P, 16], F32, kind="ExternalInput")
    qmask_d = nc.dram_tensor("qmask", [P, 4], F32, kind="ExternalInput")

    # constants baked into the NEFF (same on every core)
    rotMT_d = nc.inline_tensor(_rot_matrix_T(), name="rotMT")
    ones_d = nc.inline_tensor(np.ones((P, P), np.float32), name="onesc")

    # ---- outputs ----
    attn_o = nc.dram_tensor("attn_o", [H, QS, N], F32, kind="ExternalOutput")
    ctx_o = nc.dram_tensor("ctx_o", [QS, E], F32, kind="ExternalOutput")

    with tile.TileContext(nc) as tc, ExitStack() as ctx:
        persist = ctx.enter_context(tc.tile_pool(name="persist", bufs=1))
        khT = persist.tile([P, 4, N], F32R)       # pair p: rows = dims of heads 2p,2p+1
        qhT = persist.tile([P, 4, QS], F32R)
        vh_aug = persist.tile([P, 16, 8, 65], F32R)  # [k-chunk][128 k][head][64 d + 1]
        ctxT_sb = persist.tile([64, H, QS], F32R)    # normalized ctx^T per head
        zrec = persist.tile([P, H, 4], F32)          # 1/Z per (head, q-chunk), q on part
        rotMT = persist.tile([P, P], F32R)
        ones_r = persist.tile([P, P], F32R)   # rank-1 lhsT rows at partition 0 AND 64
        bq_sb = persist.tile([P, 4], F32)
        bk_sb = persist.tile([P, 4], F32)
        bv_sb = persist.tile([1, E], F32R)
        bo_sb = persist.tile([1, E], F32R)
        maskbias = persist.tile([P, 16], F32)
        qmask_sb = persist.tile([P, 4], F32)

        nc.sync.dma_start(out=rotMT, in_=rotMT_d[:].bitcast(F32R))
        nc.sync.dma_start(out=ones_r, in_=ones_d[:].bitcast(F32R))
        nc.sync.dma_start(out=bq_sb, in_=bq_d[:])
        nc.sync.dma_start(out=bk_sb, in_=bk_d[:])
        nc.sync.dma_start(out=bv_sb, in_=bv_d[:])
        nc.sync.dma_start(out=bo_sb, in_=bo_d[:])
        nc.sync.dma_start(out=maskbias, in_=maskbias_d[:])
        nc.sync.dma_start(out=qmask_sb, in_=qmask_d[:])
        # ones columns of vh_aug (col 64 of each head block), all k chunks
        ones_cols = bass.AP(
            tensor=ones_d.ap().tensor,
            offset=0,
            ap=[[P, P], [0, 128]],
        ).bitcast(F32R)
        nc.sync.dma_start(out=vh_aug[:, :, :, 64], in_=ones_cols)

        # ================= phase 0: projections + RoPE =================
        with (
            tc.tile_pool(name="ph0", bufs=2) as ph0,
            tc.tile_pool(name="ph0c", bufs=1) as ph0c,
        ):
            kvT_sb = ph0c.tile([P, 4, N], F32R)
            qT_sb = ph0c.tile([P, 4, QS], F32R)
            Wq_sb = ph0c.tile([P, 4, E], F32R)
            Wk_sb = ph0c.tile([P, 4, E], F32R)
            Wv_sb = ph0c.tile([P, 4, E], F32R)
            cosq = ph0c.tile([P, QS], F32)
            sinq = ph0c.tile([P, QS], F32)
            coskv = ph0c.tile([P, N], F32)
            sinkv = ph0c.tile([P, N], F32)
            nc.sync.dma_start(out=cosq, in_=cosq_d[:])
            nc.sync.dma_start(out=sinq, in_=sinq_d[:])
            nc.sync.dma_start(out=coskv, in_=coskv_d[:])
            nc.sync.dma_start(out=sinkv, in_=sinkv_d[:])
            for e in range(4):
                nc.sync.dma_start(out=kvT_sb[:, e, :], in_=kvT_d[e])
                nc.sync.dma_start(out=qT_sb[:, e, :], in_=qT_d[e])
                nc.sync.dma_start(out=Wq_sb[:, e, :], in_=Wq_d[e])
                nc.sync.dma_start(out=Wk_sb[:, e, :], in_=Wk_d[e])
                nc.sync.dma_start(out=Wv_sb[:, e, :], in_=Wv_d[e])

            # V projection: vh[kc] = kv[kc] @ Wv + bv   ([k, d] layout + Z col)
            with tc.tile_pool(name="psV", bufs=3, space="PSUM") as psV:
                for kc in range(16):
                    psv = psV.tile([P, E], F32, tag="psv")
                    for e in range(4):
                        nc.tensor.matmul(
                            psv,
                            kvT_sb[:, e, kc * P:(kc + 1) * P],
                            Wv_sb[:, e, :],
                            start=(e == 0),
                            stop=False,
                        )
                    nc.tensor.matmul(psv, ones_r[0:1, :], bv_sb, start=False, stop=True)
                    nc.vector.tensor_copy(
                        vh_aug[:, kc, :, 0:64],
                        psv.rearrange("p (h d) -> p h d", h=8),
                    )

            # Q/K projections (transposed layout) + RoPE, per head pair
            with tc.tile_pool(name="ph0ps", bufs=2, space="PSUM") as ph0ps:
                for hp in range(4):
                    psq = ph0ps.tile([P, QS], F32, tag="psq")
                    for e in range(4):
                        nc.tensor.matmul(
                            psq,
                            Wq_sb[:, e, hp * P:(hp + 1) * P],
                            qT_sb[:, e, :],
                            start=(e == 0),
                            stop=(e == 3),
                        )
                    qraw = ph0.tile([P, QS], F32R, tag="qraw")
                    nc.vector.tensor_scalar_add(qraw, psq, bq_sb[:, hp:hp + 1])
                    psrot = ph0ps.tile([P, QS], F32, tag="psrot")
                    nc.tensor.matmul(psrot, rotMT, qraw, start=True, stop=True)
                    t1 = ph0.tile([P, QS], F32, tag="t1")
                    nc.vector.tensor_mul(t1, qraw, cosq)
                    t2 = ph0.tile([P, QS], F32, tag="t2")
                    nc.vector.tensor_mul(t2, psrot, sinq)
                    nc.vector.tensor_add(qhT[:, hp, :], t1, t2)

                    for c4 in range(4):
                        psk = ph0ps.tile([P, 512], F32, tag="psk")
                        for e in range(4):
                            nc.tensor.matmul(
                                psk,
                                Wk_sb[:, e, hp * P:(hp + 1) * P],
                                kvT_sb[:, e, c4 * 512:(c4 + 1) * 512],
                                start=(e == 0),
                                stop=(e == 3),
                            )
                        kraw = ph0.tile([P, 512], F32R, tag="kraw")
                        nc.vector.tensor_scalar_add(kraw, psk, bk_sb[:, hp:hp + 1])
                        psrotk = ph0ps.tile([P, 512], F32, tag="psrotk")
                        nc.tensor.matmul(psrotk, rotMT, kraw, start=True, stop=True)
                        t1k = ph0.tile([P, 512], F32, tag="t1k")
                        nc.vector.tensor_mul(
                            t1k, kraw, coskv[:, c4 * 512:(c4 + 1) * 512])
                        t2k = ph0.tile([P, 512], F32, tag="t2k")
                        nc.vector.tensor_mul(
                            t2k, psrotk, sinkv[:, c4 * 512:(c4 + 1) * 512])
                        nc.vector.tensor_add(
                            khT[:, hp, c4 * 512:(c4 + 1) * 512], t1k, t2k)

        # ================= main loop: per head =================
        with (
            tc.tile_pool(name="mn", bufs=2) as mn,
            tc.tile_pool(name="mn1", bufs=1) as mn1,
            tc.tile_pool(name="mnsm", bufs=4) as mnsm,
            tc.tile_pool(name="psA", bufs=2, space="PSUM") as psA,
            tc.tile_pool(name="psB", bufs=2, space="PSUM") as psB,
            tc.tile_pool(name="psC", bufs=2, space="PSUM") as psC,
        ):
            # mask row replicated at partitions 0 and 64 so the rank-1 mask
            # matmul always lands in the opposite PE row-group of the K=64
            # scores matmul (they execute concurrently in the array)
            maskneg = mn1.tile([P, N], F32R, tag="maskneg")
            nc.sync.dma_start(out=maskneg[0:1, :], in_=maskneg_d[:])
            nc.sync.dma_start(out=maskneg[64:65, :], in_=maskneg_d[:])
            for h in range(H):
                hp, hs = h // 2, h % 2
                dsl = slice(hs * 64, hs * 64 + 64)

                # ---- phase A: scores [q, k], softmax, attn output ----
                for qc in range(4):
                    ptile = mn.tile([P, N], F32, tag="ptile")
                    zparts = mnsm.tile([P, 2], F32, tag="zparts")
                    for kh in range(2):
                        ps = psA.tile([P, 2, 512], F32, tag="ps_scores")
                        for kq in range(2):
                            kc4 = kh * 2 + kq
                            nc.tensor.matmul(
                                ps[:, kq, :],
                                qhT[dsl, hp, qc * P:(qc + 1) * P],
                                khT[dsl, hp, kc4 * 512:(kc4 + 1) * 512],
                                start=True,
                                stop=False,
                            )
                            row = 64 if hs == 0 else 0
                            nc.tensor.matmul(
                                ps[:, kq, :],
                                ones_r[row:row + 1, :],
                                maskneg[row:row + 1, kc4 * 512:(kc4 + 1) * 512],
                                start=False,
                                stop=True,
                            )
                        nc.scalar.activation(
                            out=ptile[:, kh * 1024:(kh + 1) * 1024],
                            in_=ps.rearrange("p a b -> p (a b)"),
                            func=mybir.ActivationFunctionType.Exp,
                            scale=SCALE,
                            accum_out=zparts[:, kh:kh + 1],
                        )
                    z1 = mnsm.tile([P, 1], F32, tag="z1")
                    nc.vector.reduce_sum(out=z1, in_=zparts, axis=mybir.AxisListType.X)
                    nc.vector.reciprocal(out=zrec[:, h, qc:qc + 1], in_=z1)
                    attn_t = mn.tile([P, N], F32, tag="attn_t")
                    nc.vector.tensor_scalar_mul(attn_t, ptile, zrec[:, h, qc:qc + 1])
                    nc.sync.dma_start(
                        out=attn_o[h, qc * P:(qc + 1) * P, :], in_=attn_t
                    )

                # ---- phase B: scores^T [k, q] -> exp(f32r) tiles for all k ----
                pT_all = mn1.tile([P, 16, 512], F32R, tag="pT_all")
                for kc in range(16):
                    psT = psB.tile([P, 512], F32, tag="psT")
                    nc.tensor.matmul(
                        psT,
                        khT[dsl, hp, kc * P:(kc + 1) * P],
                        qhT[dsl, hp, :],
                        start=True,
                        stop=True,
                    )
                    nc.scalar.activation(
                        out=pT_all[:, kc, :],
                        in_=psT,
                        func=mybir.ActivationFunctionType.Exp,
                        scale=SCALE,
                        bias=maskbias[:, kc:kc + 1],
                    )
                # ---- phase C: ctxT (+Z row) accumulation, scale by 1/Z ----
                ctxZ = psC.tile([65, QS], F32, tag="ctxZ")
                for kc in range(16):
                    nc.tensor.matmul(
                        ctxZ,
                        vh_aug[:, kc, h, :],
                        pT_all[:, kc, :],
                        start=(kc == 0),
                        stop=(kc == 15),
                    )
                zrow = mnsm.tile([1, QS], F32, tag="zrow")
                nc.vector.reciprocal(out=zrow, in_=ctxZ[64:65, :])
                zbc = mnsm.tile([64, QS], F32, tag="zbc")
                nc.gpsimd.partition_broadcast(zbc, zrow)
                nc.vector.tensor_mul(ctxT_sb[:, h, :], ctxZ[0:64, :], zbc)

        # ================= output projection =================
        with (
            tc.tile_pool(name="fin", bufs=2) as fin,
            tc.tile_pool(name="psF", bufs=2, space="PSUM") as psF,
        ):
            Wo_sb = fin.tile([64, H, E], F32R, tag="Wo_sb")
            for hh in range(H):
                nc.sync.dma_start(out=Wo_sb[:, hh, :], in_=Wo_d[hh])
            for qc in range(4):
                pso = psF.tile([P, E], F32, tag="pso")
                for hh in range(H):
                    nc.tensor.matmul(
                        pso,
                        ctxT_sb[:, hh, qc * P:(qc + 1) * P],
                        Wo_sb[:, hh, :],
                        start=(hh == 0),
                        stop=False,
                    )
                nc.tensor.matmul(pso, ones_r[0:1, :], bo_sb, start=False, stop=True)
                octx = fin.tile([P, E], F32, tag="octx")
                nc.scalar.activation(
                    out=octx, in_=pso,
                    func=mybir.ActivationFunctionType.Copy,
                    scale=qmask_sb[:, qc:qc + 1],
                )
                nc.sync.dma_start(out=ctx_o[qc * P:(qc + 1) * P, :], in_=octx)

    nc.finalize()
    return nc


def _cos_sin_tiles(pos_idx):
    """Replicates reference rope_cos_sin_from_pos + the rope_apply slicing quirk.

    Returns (cos_tile, sin_tile) each [128, T] f32: row d holds the factor for
    rotated dim d of one head (pattern repeated for the second head in a pair).
    """
    half = Dh // 2
    freqs = (1.0 / (ROPE_BASE ** (np.arange(half, dtype=np.float32) / np.float32(half)))).astype(np.float32)
    ang = pos_idx.astype(np.float32)[:, None] * freqs[None, :]       # [T, 32]
    cos_full = np.repeat(np.cos(ang).astype(np.float32), 2, axis=1)  # [T, 64]
    sin_full = np.repeat(np.sin(ang).astype(np.float32), 2, axis=1)
    c = cos_full[:, :half]                                           # [T, 32] (quirk)
    s = sin_full[:, :half]
    C = np.repeat(c, 2, axis=1).T                                    # [64, T]
    S = np.repeat(s, 2, axis=1).T
    return (
        np.ascontiguousarray(np.vstack([C, C]), dtype=np.float32),
        np.ascontiguousarray(np.vstack([S, S]), dtype=np.float32),
    )


def kernel(q, kv, q_mask, kv_mask, q_pos_idx, kv_pos_idx,
           Wq, bq, Wk, bk, Wv, bv, Wo, bo):
    q = np.asarray(q, np.float32)
    kv = np.asarray(kv, np.float32)
    q_mask = np.asarray(q_mask)
    kv_mask = np.asarray(kv_mask)
    q_pos_idx = np.asarray(q_pos_idx)
    kv_pos_idx = np.asarray(kv_pos_idx)
    Wq, Wk, Wv, Wo = (np.asarray(x, np.float32) for x in (Wq, Wk, Wv, Wo))
    bq, bk, bv, bo = (np.asarray(x, np.float32) for x in (bq, bk, bv, bo))

    if "nc" not in _NC_CACHE:
        _NC_CACHE["nc"] = build_nc()
    nc = _NC_CACHE["nc"]

    w_maps = {
        "Wq": np.ascontiguousarray(Wq.reshape(4, P, E)),
        "Wk": np.ascontiguousarray(Wk.reshape(4, P, E)),
        "Wv": np.ascontiguousarray(Wv.reshape(4, P, E)),
        "Wo": np.ascontiguousarray(Wo.reshape(H, Dh, E)),
        "bq": np.ascontiguousarray(bq.reshape(4, P).T),
        "bk": np.ascontiguousarray(bk.reshape(4, P).T),
        "bv": np.ascontiguousarray(bv[None, :]),
        "bo": np.ascontiguousarray(bo[None, :]),
    }

    in_maps = []
    for c in range(NC):
        b, qo = c // 2, (c % 2) * QS
        coskv_t, sinkv_t = _cos_sin_tiles(kv_pos_idx[b])
        cosq_t, sinq_t = _cos_sin_tiles(q_pos_idx[b, qo:qo + QS])
        mneg = np.where(kv_mask[b], 0.0, -1e9).astype(np.float32)
        m = {
            "qT": np.ascontiguousarray(q[b, qo:qo + QS, :].T.reshape(4, P, QS)),
            "kvT": np.ascontiguousarray(kv[b].T.reshape(4, P, N)),
            "cosq": cosq_t, "sinq": sinq_t,
            "coskv": coskv_t, "sinkv": sinkv_t,
            "maskneg": np.ascontiguousarray(mneg[None, :]),
            "maskbias": np.ascontiguousarray(mneg.reshape(16, P).T),
            "qmask": np.ascontiguousarray(
                q_mask[b, qo:qo + QS].astype(np.float32).reshape(4, P).T),
        }
        m.update(w_maps)
        in_maps.append(m)

    res = run_bass_kernel_spmd(nc, in_maps, core_ids=list(range(NC)))

    ctx = np.empty((B, QT, E), np.float32)
    attn = np.empty((B, H, QT, N), np.float32)
    for c in range(NC):
        b, qo = c // 2, (c % 2) * QS
        ctx[b, qo:qo + QS, :] = res.results[c]["ctx_o"]
        attn[b, :, qo:qo + QS, :] = res.results[c]["attn_o"]
    return ctx, attn


# revision 36
# speedup vs baseline: 1.0442x; 1.0410x over previous
"""Bass/Tile TRN2 kernel for nn_CrossEncoder (RoPE cross-attention, returns (ctx, attn)).

Sharding: 8 cores = (batch b, q-half). Core c handles batch c//2, query rows
[(c%2)*512, (c%2)*512+512). No collectives; each core computes its output slice:
  ctx slice  [512, 512]         -> ctx[b, qo:qo+512, :]
  attn slice [8, 512, 2048]     -> attn[b, :, qo:qo+512, :]

Matmul operands use float32r (fp32 bits, PE rounds to ~12-bit mantissa, runs at
1 cycle/row vs fp32's LOW_HIGH 2-pass) — measured 1.6e-4 matmul rel err.

Per-core pipeline:
  - projections computed transposed: qhT/khT [d-on-partition, t-on-free];
    vh in normal [k, d] layout with a ones column appended per head so the
    ctx matmul emits the softmax denominator row for free.
  - RoPE: y = x*cos + (Mrot @ x)*sin; cos/sin tables precomputed host-side.
  - scores [q,k]: K=64 matmuls + a K=1 rank-1 matmul adding -1e9 at masked k.
  - softmax: ACT exp(0.125*x) with accum_out row sums; GPSIMD normalize;
    contiguous 1MB attn stores.
  - ctx: scores^T [k,q] (mask via per-partition ACT bias) -> exp -> f32r;
    ctxT[d(+Z row), q] = vh_aug.T @ pT accumulated over k; scaled by the
    broadcast reciprocal of its own Z row; out-proj per head with K=64.
"""

import os
import sys
from contextlib import ExitStack

import numpy as np

sys.path.insert(0, "/opt/trn_rl_repo")

import concourse.bass as bass
import concourse.tile as tile
from concourse import bacc, mybir
from concourse.bass_utils import run_bass_kernel_spmd

B, QT, N, E, H, Dh = 4, 1024, 2048, 512, 8, 64
QS = 512          # q rows per core
P = 128
NC = 8
ROPE_BASE = 10000.0
SCALE = 1.0 / 8.0  # 1/sqrt(Dh)
F32 = mybir.dt.float32
F32R = mybir.dt.float32r
BF16 = mybir.dt.bfloat16

_NC_CACHE = {}


def _rot_matrix_T():
    """M s.t. (M @ x)[2p] = -x[2p+1], (M @ x)[2p+1] = x[2p]; returns M.T for lhsT."""
    M = np.zeros((P, P), np.float32)
    for p in range(P // 2):
        M[2 * p, 2 * p + 1] = -1.0
        M[2 * p + 1, 2 * p] = 1.0
    return np.ascontiguousarray(M.T)


def build_nc():
    nc = bacc.Bacc(None, target_bir_lowering=False)

    # ---- per-core external inputs (f32r = raw fp32 bits, PE rounds) ----
    qT_d = nc.dram_tensor("qT", [4, P, QS], F32R, kind="ExternalInput")
    kvT_d = nc.dram_tensor("kvT", [4, P, N], F32R, kind="ExternalInput")
    Wq_d = nc.dram_tensor("Wq", [4, P, E], F32R, kind="ExternalInput")
    Wk_d = nc.dram_tensor("Wk", [4, P, E], F32R, kind="ExternalInput")
    Wv_d = nc.dram_tensor("Wv", [4, P, E], F32R, kind="ExternalInput")
    Wo_d = nc.dram_tensor("Wo", [H, Dh, E], F32R, kind="ExternalInput")
    bq_d = nc.dram_tensor("bq", [P, 4], F32, kind="ExternalInput")
    bk_d = nc.dram_tensor("bk", [P, 4], F32, kind="ExternalInput")
    bv_d = nc.dram_tensor("bv", [1, E], F32R, kind="ExternalInput")
    bo_d = nc.dram_tensor("bo", [1, E], F32R, kind="ExternalInput")
    cosq_d = nc.dram_tensor("cosq", [P, QS], F32, kind="ExternalInput")
    sinq_d = nc.dram_tensor("sinq", [P, QS], F32, kind="ExternalInput")
    coskv_d = nc.dram_tensor("coskv", [P, N], F32, kind="ExternalInput")
    sinkv_d = nc.dram_tensor("sinkv", [P, N], F32, kind="ExternalInput")
    maskneg_d = nc.dram_tensor("maskneg", [1, N], F32R, kind="ExternalInput")
    maskbias_d = nc.dram_tensor("maskbias", [P, 16], F32, kind="ExternalInput")
    qmask_d = nc.dram_tensor("qmask", [P, 4], F32, kind="ExternalInput")

    # constants baked into the NEFF (same on every core)
    rotMT_d = nc.inline_tensor(_rot_matrix_T(), name="rotMT")
    ones_d = nc.inline_tensor(np.ones((P, P), np.float32), name="onesc")

    # ---- outputs ----
    attn_o = nc.dram_tensor("attn_o", [H, QS, N], F32, kind="ExternalOutput")
    ctx_o = nc.dram_tensor("ctx_o", [QS, E], F32, kind="ExternalOutput")

    with tile.TileContext(nc) as tc, ExitStack() as ctx:
        persist = ctx.enter_context(tc.tile_pool(name="persist", bufs=1))
        khT = persist.tile([P, 4, N], F32R)       # pair p: rows = dims of heads 2p,2p+1
        qhT = persist.tile([P, 4, QS], F32R)
        vh_aug = persist.tile([P, 16, 8, 65], BF16)  # [k-chunk][128 k][head][64 d + 1]
        ctxT_sb = persist.tile([64, H, QS], F32R)    # normalized ctx^T per head
        zrec = persist.tile([P, H, 4], F32)          # 1/Z per (head, q-chunk), q on part
        rotMT = persist.tile([P, P], F32R)
        ones_r = persist.tile([P, P], F32R)   # rank-1 lhsT rows at partition 0 AND 64
        bq_sb = persist.tile([P, 4], F32)
        bk_sb = persist.tile([P, 4], F32)
        bv_sb = persist.tile([1, E], F32R)
        bo_sb = persist.tile([1, E], F32R)
        maskbias = persist.tile([P, 16], F32)
        qmask_sb = persist.tile([P, 4], F32)

        nc.sync.dma_start(out=rotMT, in_=rotMT_d[:].bitcast(F32R))
        nc.sync.dma_start(out=ones_r, in_=ones_d[:].bitcast(F32R))
        nc.sync.dma_start(out=bq_sb, in_=bq_d[:])
        nc.sync.dma_start(out=bk_sb, in_=bk_d[:])
        nc.sync.dma_start(out=bv_sb, in_=bv_d[:])
        nc.sync.dma_start(out=bo_sb, in_=bo_d[:])
        nc.sync.dma_start(out=maskbias, in_=maskbias_d[:])
        nc.sync.dma_start(out=qmask_sb, in_=qmask_d[:])
        # ones columns of vh_aug (col 64 of each head block), all k chunks
        nc.vector.memset(vh_aug[:, :, :, 64], 1.0)

        # ================= phase 0: projections + RoPE =================
        with (
            tc.tile_pool(name="ph0", bufs=2) as ph0,
            tc.tile_pool(name="ph0c", bufs=1) as ph0c,
        ):
            kvT_sb = ph0c.tile([P, 4, N], F32R)
            qT_sb = ph0c.tile([P, 4, QS], F32R)
            Wq_sb = ph0c.tile([P, 4, E], F32R)
            Wk_sb = ph0c.tile([P, 4, E], F32R)
            Wv_sb = ph0c.tile([P, 4, E], F32R)
            cosq = ph0c.tile([P, QS], F32)
            sinq = ph0c.tile([P, QS], F32)
            coskv = ph0c.tile([P, N], F32)
            sinkv = ph0c.tile([P, N], F32)
            nc.sync.dma_start(out=cosq, in_=cosq_d[:])
            nc.sync.dma_start(out=sinq, in_=sinq_d[:])
            nc.sync.dma_start(out=coskv, in_=coskv_d[:])
            nc.sync.dma_start(out=sinkv, in_=sinkv_d[:])
            for e in range(4):
                nc.sync.dma_start(out=kvT_sb[:, e, :], in_=kvT_d[e])
                nc.sync.dma_start(out=qT_sb[:, e, :], in_=qT_d[e])
                nc.sync.dma_start(out=Wq_sb[:, e, :], in_=Wq_d[e])
                nc.sync.dma_start(out=Wk_sb[:, e, :], in_=Wk_d[e])
                nc.sync.dma_start(out=Wv_sb[:, e, :], in_=Wv_d[e])

            # V projection: vh[kc] = kv[kc] @ Wv + bv   ([k, d] layout + Z col)
            with tc.tile_pool(name="psV", bufs=3, space="PSUM") as psV:
                for kc in range(16):
                    psv = psV.tile([P, E], F32, tag="psv")
                    for e in range(4):
                        nc.tensor.matmul(
                            psv,
                            kvT_sb[:, e, kc * P:(kc + 1) * P],
                            Wv_sb[:, e, :],
                            start=(e == 0),
                            stop=False,
                        )
                    nc.tensor.matmul(psv, ones_r[0:1, :], bv_sb, start=False, stop=True)
                    nc.vector.tensor_copy(
                        vh_aug[:, kc, :, 0:64],
                        psv.rearrange("p (h d) -> p h d", h=8),
                    )

            # Q/K projections (transposed layout) + RoPE, per head pair
            with tc.tile_pool(name="ph0ps", bufs=2, space="PSUM") as ph0ps:
                for hp in range(4):
                    psq = ph0ps.tile([P, QS], F32, tag="psq")
                    for e in range(4):
                        nc.tensor.matmul(
                            psq,
                            Wq_sb[:, e, hp * P:(hp + 1) * P],
                            qT_sb[:, e, :],
                            start=(e == 0),
                            stop=(e == 3),
                        )
                    qraw = ph0.tile([P, QS], F32R, tag="qraw")
                    nc.vector.tensor_scalar_add(qraw, psq, bq_sb[:, hp:hp + 1])
                    psrot = ph0ps.tile([P, QS], F32, tag="psrot")
                    nc.tensor.matmul(psrot, rotMT, qraw, start=True, stop=True)
                    t1 = ph0.tile([P, QS], F32, tag="t1")
                    nc.vector.tensor_mul(t1, qraw, cosq)
                    t2 = ph0.tile([P, QS], F32, tag="t2")
                    nc.vector.tensor_mul(t2, psrot, sinq)
                    nc.vector.tensor_add(qhT[:, hp, :], t1, t2)

                    for c4 in range(4):
                        psk = ph0ps.tile([P, 512], F32, tag="psk")
                        for e in range(4):
                            nc.tensor.matmul(
                                psk,
                                Wk_sb[:, e, hp * P:(hp + 1) * P],
                                kvT_sb[:, e, c4 * 512:(c4 + 1) * 512],
                                start=(e == 0),
                                stop=(e == 3),
                            )
                        kraw = ph0.tile([P, 512], F32R, tag="kraw")
                        nc.vector.tensor_scalar_add(kraw, psk, bk_sb[:, hp:hp + 1])
                        psrotk = ph0ps.tile([P, 512], F32, tag="psrotk")
                        nc.tensor.matmul(psrotk, rotMT, kraw, start=True, stop=True)
                        t1k = ph0.tile([P, 512], F32, tag="t1k")
                        nc.vector.tensor_mul(
                            t1k, kraw, coskv[:, c4 * 512:(c4 + 1) * 512])
                        t2k = ph0.tile([P, 512], F32, tag="t2k")
                        nc.vector.tensor_mul(
                            t2k, psrotk, sinkv[:, c4 * 512:(c4 + 1) * 512])
                        nc.vector.tensor_add(
                            khT[:, hp, c4 * 512:(c4 + 1) * 512], t1k, t2k)

        # ================= main loop: per head =================
        with (
            tc.tile_pool(name="mn", bufs=2) as mn,
            tc.tile_pool(name="mn1", bufs=2) as mn1,
            tc.tile_pool(name="mnsm", bufs=4) as mnsm,
            tc.tile_pool(name="psA", bufs=2, space="PSUM") as psA,
            tc.tile_pool(name="psB", bufs=2, space="PSUM") as psB,
            tc.tile_pool(name="psC", bufs=2, space="PSUM") as psC,
        ):
            # mask row replicated at partitions 0 and 64 so the rank-1 mask
            # matmul always lands in the opposite PE row-group of the K=64
            # scores matmul (they execute concurrently in the array)
            maskneg = mn1.tile([P, N], F32R, tag="maskneg")
            nc.sync.dma_start(out=maskneg[0:1, :], in_=maskneg_d[:])
            nc.sync.dma_start(out=maskneg[64:65, :], in_=maskneg_d[:])

            def phase_c(h, pT_all):
                # ctxT (+Z row) accumulation for head h, scale by 1/Z
                ctxZ = psC.tile([65, QS], F32, tag="ctxZ")
                for kc in range(16):
                    nc.tensor.matmul(
                        ctxZ,
                        vh_aug[:, kc, h, :],
                        pT_all[:, kc, :],
                        start=(kc == 0),
                        stop=(kc == 15),
                    )
                zrow = mnsm.tile([1, QS], F32, tag="zrow")
                nc.vector.reciprocal(out=zrow, in_=ctxZ[64:65, :])
                zbc = mnsm.tile([64, QS], F32, tag="zbc")
                nc.gpsimd.partition_broadcast(zbc, zrow)
                nc.vector.tensor_mul(ctxT_sb[:, h, :], ctxZ[0:64, :], zbc)

            prev = None
            for h in range(H):
                hp, hs = h // 2, h % 2
                dsl = slice(hs * 64, hs * 64 + 64)

                # ---- phase A: scores [q, k], softmax, attn output ----
                for qc in range(4):
                    ptile = mn.tile([P, N], F32, tag="ptile")
                    zparts = mnsm.tile([P, 2], F32, tag="zparts")
                    for kh in range(2):
                        ps = psA.tile([P, 2, 512], F32, tag="ps_scores")
                        for kq in range(2):
                            kc4 = kh * 2 + kq
                            nc.tensor.matmul(
                                ps[:, kq, :],
                                qhT[dsl, hp, qc * P:(qc + 1) * P],
                                khT[dsl, hp, kc4 * 512:(kc4 + 1) * 512],
                                start=True,
                                stop=False,
                            )
                            row = 64 if hs == 0 else 0
                            nc.tensor.matmul(
                                ps[:, kq, :],
                                ones_r[row:row + 1, :],
                                maskneg[row:row + 1, kc4 * 512:(kc4 + 1) * 512],
                                start=False,
                                stop=True,
                            )
                        nc.scalar.activation(
                            out=ptile[:, kh * 1024:(kh + 1) * 1024],
                            in_=ps.rearrange("p a b -> p (a b)"),
                            func=mybir.ActivationFunctionType.Exp,
                            scale=SCALE,
                            accum_out=zparts[:, kh:kh + 1],
                        )
                    z1 = mnsm.tile([P, 1], F32, tag="z1")
                    nc.vector.reduce_sum(out=z1, in_=zparts, axis=mybir.AxisListType.X)
                    nc.vector.reciprocal(out=zrec[:, h, qc:qc + 1], in_=z1)
                    attn_t = mn.tile([P, N], F32, tag="attn_t")
                    nc.vector.tensor_scalar_mul(attn_t, ptile, zrec[:, h, qc:qc + 1])
                    nc.sync.dma_start(
                        out=attn_o[h, qc * P:(qc + 1) * P, :], in_=attn_t
                    )

                # ---- phase B: scores^T [k, q] -> exp(bf16) tiles for all k ----
                pT_all = mn1.tile([P, 16, 512], BF16, tag="pT_all")
                for kc in range(16):
                    psT = psB.tile([P, 512], F32, tag="psT")
                    nc.tensor.matmul(
                        psT,
                        khT[dsl, hp, kc * P:(kc + 1) * P],
                        qhT[dsl, hp, :],
                        start=True,
                        stop=True,
                    )
                    nc.scalar.activation(
                        out=pT_all[:, kc, :],
                        in_=psT,
                        func=mybir.ActivationFunctionType.Exp,
                        scale=SCALE,
                        bias=maskbias[:, kc:kc + 1],
                    )
                # ---- phase C pipelined one head behind: PE never waits on ACT ----
                if prev is not None:
                    phase_c(*prev)
                prev = (h, pT_all)
            phase_c(*prev)

        # ================= output projection =================
        with (
            tc.tile_pool(name="fin", bufs=2) as fin,
            tc.tile_pool(name="psF", bufs=2, space="PSUM") as psF,
        ):
            Wo_sb = fin.tile([64, H, E], F32R, tag="Wo_sb")
            for hh in range(H):
                nc.sync.dma_start(out=Wo_sb[:, hh, :], in_=Wo_d[hh])
            for qc in range(4):
                pso = psF.tile([P, E], F32, tag="pso")
                for hh in range(H):
                    nc.tensor.matmul(
                        pso,
                        ctxT_sb[:, hh, qc * P:(qc + 1) * P],
                        Wo_sb[:, hh, :],
                        start=(hh == 0),
                        stop=False,
                    )
                nc.tensor.matmul(pso, ones_r[0:1, :], bo_sb, start=False, stop=True)
                octx = fin.tile([P, E], F32, tag="octx")
                nc.scalar.activation(
                    out=octx, in_=pso,
                    func=mybir.ActivationFunctionType.Copy,
                    scale=qmask_sb[:, qc:qc + 1],
                )
                nc.sync.dma_start(out=ctx_o[qc * P:(qc + 1) * P, :], in_=octx)

    nc.finalize()
    return nc


def _cos_sin_tiles(pos_idx):
    """Replicates reference rope_cos_sin_from_pos + the rope_apply slicing quirk.

    Returns (cos_tile, sin_tile) each [128, T] f32: row d holds the factor for
    rotated dim d of one head (pattern repeated for the second head in a pair).
    """
    half = Dh // 2
    freqs = (1.0 / (ROPE_BASE ** (np.arange(half, dtype=np.float32) / np.float32(half)))).astype(np.float32)
    ang = pos_idx.astype(np.float32)[:, None] * freqs[None, :]       # [T, 32]
    cos_full = np.repeat(np.cos(ang).astype(np.float32), 2, axis=1)  # [T, 64]
    sin_full = np.repeat(np.sin(ang).astype(np.float32), 2, axis=1)
    c = cos_full[:, :half]                                           # [T, 32] (quirk)
    s = sin_full[:, :half]
    C = np.repeat(c, 2, axis=1).T                                    # [64, T]
    S = np.repeat(s, 2, axis=1).T
    return (
        np.ascontiguousarray(np.vstack([C, C]), dtype=np.float32),
        np.ascontiguousarray(np.vstack([S, S]), dtype=np.float32),
    )


def kernel(q, kv, q_mask, kv_mask, q_pos_idx, kv_pos_idx,
           Wq, bq, Wk, bk, Wv, bv, Wo, bo):
    q = np.asarray(q, np.float32)
    kv = np.asarray(kv, np.float32)
    q_mask = np.asarray(q_mask)
    kv_mask = np.asarray(kv_mask)
    q_pos_idx = np.asarray(q_pos_idx)
    kv_pos_idx = np.asarray(kv_pos_idx)
    Wq, Wk, Wv, Wo = (np.asarray(x, np.float32) for x in (Wq, Wk, Wv, Wo))
    bq, bk, bv, bo = (np.asarray(x, np.float32) for x in (bq, bk, bv, bo))

    if "nc" not in _NC_CACHE:
        _NC_CACHE["nc"] = build_nc()
    nc = _NC_CACHE["nc"]

    w_maps = {
        "Wq": np.ascontiguousarray(Wq.reshape(4, P, E)),
        "Wk": np.ascontiguousarray(Wk.reshape(4, P, E)),
        "Wv": np.ascontiguousarray(Wv.reshape(4, P, E)),
        "Wo": np.ascontiguousarray(Wo.reshape(H, Dh, E)),
        "bq": np.ascontiguousarray(bq.reshape(4, P).T),
        "bk": np.ascontiguousarray(bk.reshape(4, P).T),
        "bv": np.ascontiguousarray(bv[None, :]),
        "bo": np.ascontiguousarray(bo[None, :]),
    }

    in_maps = []
    for c in range(NC):
        b, qo = c // 2, (c % 2) * QS
        coskv_t, sinkv_t = _cos_sin_tiles(kv_pos_idx[b])
        cosq_t, sinq_t = _cos_sin_tiles(q_pos_idx[b, qo:qo + QS])
        mneg = np.where(kv_mask[b], 0.0, -1e9).astype(np.float32)
        m = {
            "qT": np.ascontiguousarray(q[b, qo:qo + QS, :].T.reshape(4, P, QS)),
            "kvT": np.ascontiguousarray(kv[b].T.reshape(4, P, N)),
            "cosq": cosq_t, "sinq": sinq_t,
            "coskv": coskv_t, "sinkv": sinkv_t,
            "maskneg": np.ascontiguousarray(mneg[None, :]),
            "maskbias": np.ascontiguousarray(mneg.reshape(16, P).T),
            "qmask": np.ascontiguousarray(
                q_mask[b, qo:qo + QS].astype(np.float32).reshape(4, P).T),
        }
        m.update(w_maps)
        in_maps.append(m)

    res = run_bass_kernel_spmd(nc, in_maps, core_ids=list(range(NC)))

    ctx = np.empty((B, QT, E), np.float32)
    attn = np.empty((B, H, QT, N), np.float32)
    for c in range(NC):
        b, qo = c // 2, (c % 2) * QS
        ctx[b, qo:qo + QS, :] = res.results[c]["ctx_o"]
        attn[b, :, qo:qo + QS, :] = res.results[c]["attn_o"]
    return ctx, attn


# revision 37
# speedup vs baseline: 1.3050x; 1.2497x over previous
"""Bass/Tile TRN2 kernel for nn_CrossEncoder (RoPE cross-attention, returns (ctx, attn)).

Sharding: 8 cores = (batch b, q-half). Core c handles batch c//2, query rows
[(c%2)*512, (c%2)*512+512). No collectives; each core computes its output slice:
  ctx slice  [512, 512]         -> ctx[b, qo:qo+512, :]
  attn slice [8, 512, 2048]     -> attn[b, :, qo:qo+512, :]

Matmul operands use float32r (fp32 bits, PE rounds to ~12-bit mantissa, runs at
1 cycle/row vs fp32's LOW_HIGH 2-pass) — measured 1.6e-4 matmul rel err.

Per-core pipeline:
  - projections computed transposed: qhT/khT [d-on-partition, t-on-free];
    vh in normal [k, d] layout with a ones column appended per head so the
    ctx matmul emits the softmax denominator row for free.
  - RoPE: y = x*cos + (Mrot @ x)*sin; cos/sin tables precomputed host-side.
  - scores [q,k]: K=64 matmuls + a K=1 rank-1 matmul adding -1e9 at masked k.
  - softmax: ACT exp(0.125*x) with accum_out row sums; GPSIMD normalize;
    contiguous 1MB attn stores.
  - ctx: scores^T [k,q] (mask via per-partition ACT bias) -> exp -> f32r;
    ctxT[d(+Z row), q] = vh_aug.T @ pT accumulated over k; scaled by the
    broadcast reciprocal of its own Z row; out-proj per head with K=64.
"""

import os
import sys
from contextlib import ExitStack

import numpy as np

sys.path.insert(0, "/opt/trn_rl_repo")

import ml_dtypes

import concourse.bass as bass
import concourse.tile as tile
from concourse import bacc, mybir
from concourse.bass_utils import run_bass_kernel_spmd

B, QT, N, E, H, Dh = 4, 1024, 2048, 512, 8, 64
QS = 512          # q rows per core
P = 128
NC = 8
ROPE_BASE = 10000.0
SCALE = 1.0 / 8.0  # 1/sqrt(Dh)
F32 = mybir.dt.float32
F32R = mybir.dt.float32r
BF16 = mybir.dt.bfloat16

_NC_CACHE = {}


def _rot_matrix_T():
    """M s.t. (M @ x)[2p] = -x[2p+1], (M @ x)[2p+1] = x[2p]; returns M.T for lhsT."""
    M = np.zeros((P, P), np.float32)
    for p in range(P // 2):
        M[2 * p, 2 * p + 1] = -1.0
        M[2 * p + 1, 2 * p] = 1.0
    return np.ascontiguousarray(M.T)


def build_nc():
    nc = bacc.Bacc(None, target_bir_lowering=False)

    # ---- per-core external inputs (f32r = raw fp32 bits, PE rounds) ----
    qT_d = nc.dram_tensor("qT", [4, P, QS], BF16, kind="ExternalInput")
    kvT_d = nc.dram_tensor("kvT", [4, P, N], BF16, kind="ExternalInput")
    Wq_d = nc.dram_tensor("Wq", [4, P, E], BF16, kind="ExternalInput")
    Wk_d = nc.dram_tensor("Wk", [4, P, E], BF16, kind="ExternalInput")
    Wv_d = nc.dram_tensor("Wv", [4, P, E], BF16, kind="ExternalInput")
    Wo_d = nc.dram_tensor("Wo", [H, Dh, E], BF16, kind="ExternalInput")
    bq_d = nc.dram_tensor("bq", [P, 4], F32, kind="ExternalInput")
    bk_d = nc.dram_tensor("bk", [P, 4], F32, kind="ExternalInput")
    bv_d = nc.dram_tensor("bv", [1, E], BF16, kind="ExternalInput")
    bo_d = nc.dram_tensor("bo", [1, E], BF16, kind="ExternalInput")
    cosq_d = nc.dram_tensor("cosq", [P, QS], F32, kind="ExternalInput")
    sinq_d = nc.dram_tensor("sinq", [P, QS], F32, kind="ExternalInput")
    coskv_d = nc.dram_tensor("coskv", [P, N], F32, kind="ExternalInput")
    sinkv_d = nc.dram_tensor("sinkv", [P, N], F32, kind="ExternalInput")
    maskneg_d = nc.dram_tensor("maskneg", [1, N], BF16, kind="ExternalInput")
    maskbias_d = nc.dram_tensor("maskbias", [P, 16], F32, kind="ExternalInput")
    qmask_d = nc.dram_tensor("qmask", [P, 4], F32, kind="ExternalInput")

    # constants baked into the NEFF (same on every core)
    rotMT_d = nc.inline_tensor(_rot_matrix_T().astype(ml_dtypes.bfloat16), name="rotMT")
    ones_d = nc.inline_tensor(np.ones((P, P), ml_dtypes.bfloat16), name="onesc")

    # ---- outputs ----
    attn_o = nc.dram_tensor("attn_o", [H, QS, N], F32, kind="ExternalOutput")
    ctx_o = nc.dram_tensor("ctx_o", [QS, E], F32, kind="ExternalOutput")

    with tile.TileContext(nc) as tc, ExitStack() as ctx:
        persist = ctx.enter_context(tc.tile_pool(name="persist", bufs=1))
        khT = persist.tile([P, 4, N], BF16)       # pair p: rows = dims of heads 2p,2p+1
        qhT = persist.tile([P, 4, QS], BF16)
        vh_aug = persist.tile([P, 16, 8, 65], BF16)  # [k-chunk][128 k][head][64 d + 1]
        ctxT_sb = persist.tile([64, H, QS], BF16)    # normalized ctx^T per head
        zrec = persist.tile([P, H, 4], F32)          # 1/Z per (head, q-chunk), q on part
        rotMT = persist.tile([P, P], BF16)
        ones_r = persist.tile([P, P], BF16)   # rank-1 lhsT rows at partition 0 AND 64
        bq_sb = persist.tile([P, 4], F32)
        bk_sb = persist.tile([P, 4], F32)
        bv_sb = persist.tile([1, E], BF16)
        bo_sb = persist.tile([1, E], BF16)
        maskbias = persist.tile([P, 16], F32)
        qmask_sb = persist.tile([P, 4], F32)

        nc.sync.dma_start(out=rotMT, in_=rotMT_d[:])
        nc.sync.dma_start(out=ones_r, in_=ones_d[:])
        nc.sync.dma_start(out=bq_sb, in_=bq_d[:])
        nc.sync.dma_start(out=bk_sb, in_=bk_d[:])
        nc.sync.dma_start(out=bv_sb, in_=bv_d[:])
        nc.sync.dma_start(out=bo_sb, in_=bo_d[:])
        nc.sync.dma_start(out=maskbias, in_=maskbias_d[:])
        nc.sync.dma_start(out=qmask_sb, in_=qmask_d[:])
        # ones columns of vh_aug (col 64 of each head block), all k chunks
        nc.vector.memset(vh_aug[:, :, :, 64], 1.0)

        # ================= phase 0: projections + RoPE =================
        with (
            tc.tile_pool(name="ph0", bufs=2) as ph0,
            tc.tile_pool(name="ph0c", bufs=1) as ph0c,
        ):
            kvT_sb = ph0c.tile([P, 4, N], BF16)
            qT_sb = ph0c.tile([P, 4, QS], BF16)
            Wq_sb = ph0c.tile([P, 4, E], BF16)
            Wk_sb = ph0c.tile([P, 4, E], BF16)
            Wv_sb = ph0c.tile([P, 4, E], BF16)
            cosq = ph0c.tile([P, QS], F32)
            sinq = ph0c.tile([P, QS], F32)
            coskv = ph0c.tile([P, N], F32)
            sinkv = ph0c.tile([P, N], F32)
            nc.sync.dma_start(out=cosq, in_=cosq_d[:])
            nc.sync.dma_start(out=sinq, in_=sinq_d[:])
            nc.sync.dma_start(out=coskv, in_=coskv_d[:])
            nc.sync.dma_start(out=sinkv, in_=sinkv_d[:])
            for e in range(4):
                nc.sync.dma_start(out=kvT_sb[:, e, :], in_=kvT_d[e])
                nc.sync.dma_start(out=qT_sb[:, e, :], in_=qT_d[e])
                nc.sync.dma_start(out=Wq_sb[:, e, :], in_=Wq_d[e])
                nc.sync.dma_start(out=Wk_sb[:, e, :], in_=Wk_d[e])
                nc.sync.dma_start(out=Wv_sb[:, e, :], in_=Wv_d[e])

            # V projection: vh[kc] = kv[kc] @ Wv + bv   ([k, d] layout + Z col)
            with tc.tile_pool(name="psV", bufs=3, space="PSUM") as psV:
                for kc in range(16):
                    psv = psV.tile([P, E], F32, tag="psv")
                    for e in range(4):
                        nc.tensor.matmul(
                            psv,
                            kvT_sb[:, e, kc * P:(kc + 1) * P],
                            Wv_sb[:, e, :],
                            start=(e == 0),
                            stop=False,
                        )
                    nc.tensor.matmul(psv, ones_r[0:1, :], bv_sb, start=False, stop=True)
                    nc.vector.tensor_copy(
                        vh_aug[:, kc, :, 0:64],
                        psv.rearrange("p (h d) -> p h d", h=8),
                    )

            # Q/K projections (transposed layout) + RoPE, per head pair
            with tc.tile_pool(name="ph0ps", bufs=2, space="PSUM") as ph0ps:
                for hp in range(4):
                    psq = ph0ps.tile([P, QS], F32, tag="psq")
                    for e in range(4):
                        nc.tensor.matmul(
                            psq,
                            Wq_sb[:, e, hp * P:(hp + 1) * P],
                            qT_sb[:, e, :],
                            start=(e == 0),
                            stop=(e == 3),
                        )
                    qraw = ph0.tile([P, QS], BF16, tag="qraw")
                    nc.vector.tensor_scalar_add(qraw, psq, bq_sb[:, hp:hp + 1])
                    psrot = ph0ps.tile([P, QS], F32, tag="psrot")
                    nc.tensor.matmul(psrot, rotMT, qraw, start=True, stop=True)
                    t1 = ph0.tile([P, QS], F32, tag="t1")
                    nc.vector.tensor_mul(t1, qraw, cosq)
                    t2 = ph0.tile([P, QS], F32, tag="t2")
                    nc.vector.tensor_mul(t2, psrot, sinq)
                    nc.vector.tensor_add(qhT[:, hp, :], t1, t2)

                    for c4 in range(4):
                        psk = ph0ps.tile([P, 512], F32, tag="psk")
                        for e in range(4):
                            nc.tensor.matmul(
                                psk,
                                Wk_sb[:, e, hp * P:(hp + 1) * P],
                                kvT_sb[:, e, c4 * 512:(c4 + 1) * 512],
                                start=(e == 0),
                                stop=(e == 3),
                            )
                        kraw = ph0.tile([P, 512], BF16, tag="kraw")
                        nc.vector.tensor_scalar_add(kraw, psk, bk_sb[:, hp:hp + 1])
                        psrotk = ph0ps.tile([P, 512], F32, tag="psrotk")
                        nc.tensor.matmul(psrotk, rotMT, kraw, start=True, stop=True)
                        t1k = ph0.tile([P, 512], F32, tag="t1k")
                        nc.vector.tensor_mul(
                            t1k, kraw, coskv[:, c4 * 512:(c4 + 1) * 512])
                        t2k = ph0.tile([P, 512], F32, tag="t2k")
                        nc.vector.tensor_mul(
                            t2k, psrotk, sinkv[:, c4 * 512:(c4 + 1) * 512])
                        nc.vector.tensor_add(
                            khT[:, hp, c4 * 512:(c4 + 1) * 512], t1k, t2k)

        # ================= main loop: per head =================
        with (
            tc.tile_pool(name="mn", bufs=2) as mn,
            tc.tile_pool(name="mn1", bufs=2) as mn1,
            tc.tile_pool(name="mnsm", bufs=4) as mnsm,
            tc.tile_pool(name="psA", bufs=2, space="PSUM") as psA,
            tc.tile_pool(name="psB", bufs=2, space="PSUM") as psB,
            tc.tile_pool(name="psC", bufs=2, space="PSUM") as psC,
        ):
            # mask row replicated at partitions 0 and 64 so the rank-1 mask
            # matmul always lands in the opposite PE row-group of the K=64
            # scores matmul (they execute concurrently in the array)
            maskneg = mn1.tile([P, N], BF16, tag="maskneg")
            nc.sync.dma_start(out=maskneg[0:1, :], in_=maskneg_d[:])
            nc.sync.dma_start(out=maskneg[64:65, :], in_=maskneg_d[:])

            def phase_c(h, pT_all):
                # ctxT (+Z row) accumulation for head h, scale by 1/Z
                ctxZ = psC.tile([65, QS], F32, tag="ctxZ")
                for kc in range(16):
                    nc.tensor.matmul(
                        ctxZ,
                        vh_aug[:, kc, h, :],
                        pT_all[:, kc, :],
                        start=(kc == 0),
                        stop=(kc == 15),
                    )
                zrow = mnsm.tile([1, QS], F32, tag="zrow")
                nc.vector.reciprocal(out=zrow, in_=ctxZ[64:65, :])
                zbc = mnsm.tile([64, QS], F32, tag="zbc")
                nc.gpsimd.partition_broadcast(zbc, zrow)
                nc.vector.tensor_mul(ctxT_sb[:, h, :], ctxZ[0:64, :], zbc)

            prev = None
            for h in range(H):
                hp, hs = h // 2, h % 2
                dsl = slice(hs * 64, hs * 64 + 64)

                # ---- phase A: scores [q, k], softmax, attn output ----
                for qc in range(4):
                    ptile = mn.tile([P, N], F32, tag="ptile")
                    zparts = mnsm.tile([P, 2], F32, tag="zparts")
                    for kh in range(2):
                        ps = psA.tile([P, 2, 512], F32, tag="ps_scores")
                        for kq in range(2):
                            kc4 = kh * 2 + kq
                            nc.tensor.matmul(
                                ps[:, kq, :],
                                qhT[dsl, hp, qc * P:(qc + 1) * P],
                                khT[dsl, hp, kc4 * 512:(kc4 + 1) * 512],
                                start=True,
                                stop=False,
                            )
                            row = 64 if hs == 0 else 0
                            nc.tensor.matmul(
                                ps[:, kq, :],
                                ones_r[row:row + 1, :],
                                maskneg[row:row + 1, kc4 * 512:(kc4 + 1) * 512],
                                start=False,
                                stop=True,
                            )
                        nc.scalar.activation(
                            out=ptile[:, kh * 1024:(kh + 1) * 1024],
                            in_=ps.rearrange("p a b -> p (a b)"),
                            func=mybir.ActivationFunctionType.Exp,
                            scale=SCALE,
                            accum_out=zparts[:, kh:kh + 1],
                        )
                    z1 = mnsm.tile([P, 1], F32, tag="z1")
                    nc.vector.reduce_sum(out=z1, in_=zparts, axis=mybir.AxisListType.X)
                    nc.vector.reciprocal(out=zrec[:, h, qc:qc + 1], in_=z1)
                    attn_t = mn.tile([P, N], F32, tag="attn_t")
                    nc.vector.tensor_scalar_mul(attn_t, ptile, zrec[:, h, qc:qc + 1])
                    nc.sync.dma_start(
                        out=attn_o[h, qc * P:(qc + 1) * P, :], in_=attn_t
                    )

                # ---- phase B: scores^T [k, q] -> exp(bf16) tiles for all k ----
                pT_all = mn1.tile([P, 16, 512], BF16, tag="pT_all")
                for kc in range(16):
                    psT = psB.tile([P, 512], F32, tag="psT")
                    nc.tensor.matmul(
                        psT,
                        khT[dsl, hp, kc * P:(kc + 1) * P],
                        qhT[dsl, hp, :],
                        start=True,
                        stop=True,
                    )
                    nc.scalar.activation(
                        out=pT_all[:, kc, :],
                        in_=psT,
                        func=mybir.ActivationFunctionType.Exp,
                        scale=SCALE,
                        bias=maskbias[:, kc:kc + 1],
                    )
                # ---- phase C pipelined one head behind: PE never waits on ACT ----
                if prev is not None:
                    phase_c(*prev)
                prev = (h, pT_all)
            phase_c(*prev)

        # ================= output projection =================
        with (
            tc.tile_pool(name="fin", bufs=2) as fin,
            tc.tile_pool(name="psF", bufs=2, space="PSUM") as psF,
        ):
            Wo_sb = fin.tile([64, H, E], BF16, tag="Wo_sb")
            for hh in range(H):
                nc.sync.dma_start(out=Wo_sb[:, hh, :], in_=Wo_d[hh])
            for qc in range(4):
                pso = psF.tile([P, E], F32, tag="pso")
                for hh in range(H):
                    nc.tensor.matmul(
                        pso,
                        ctxT_sb[:, hh, qc * P:(qc + 1) * P],
                        Wo_sb[:, hh, :],
                        start=(hh == 0),
                        stop=False,
                    )
                nc.tensor.matmul(pso, ones_r[0:1, :], bo_sb, start=False, stop=True)
                octx = fin.tile([P, E], F32, tag="octx")
                nc.scalar.activation(
                    out=octx, in_=pso,
                    func=mybir.ActivationFunctionType.Copy,
                    scale=qmask_sb[:, qc:qc + 1],
                )
                nc.sync.dma_start(out=ctx_o[qc * P:(qc + 1) * P, :], in_=octx)

    nc.finalize()
    return nc


def _cos_sin_tiles(pos_idx):
    """Replicates reference rope_cos_sin_from_pos + the rope_apply slicing quirk.

    Returns (cos_tile, sin_tile) each [128, T] f32: row d holds the factor for
    rotated dim d of one head (pattern repeated for the second head in a pair).
    """
    half = Dh // 2
    freqs = (1.0 / (ROPE_BASE ** (np.arange(half, dtype=np.float32) / np.float32(half)))).astype(np.float32)
    ang = pos_idx.astype(np.float32)[:, None] * freqs[None, :]       # [T, 32]
    cos_full = np.repeat(np.cos(ang).astype(np.float32), 2, axis=1)  # [T, 64]
    sin_full = np.repeat(np.sin(ang).astype(np.float32), 2, axis=1)
    c = cos_full[:, :half]                                           # [T, 32] (quirk)
    s = sin_full[:, :half]
    C = np.repeat(c, 2, axis=1).T                                    # [64, T]
    S = np.repeat(s, 2, axis=1).T
    return (
        np.ascontiguousarray(np.vstack([C, C]), dtype=np.float32),
        np.ascontiguousarray(np.vstack([S, S]), dtype=np.float32),
    )


def kernel(q, kv, q_mask, kv_mask, q_pos_idx, kv_pos_idx,
           Wq, bq, Wk, bk, Wv, bv, Wo, bo):
    q = np.asarray(q, np.float32)
    kv = np.asarray(kv, np.float32)
    q_mask = np.asarray(q_mask)
    kv_mask = np.asarray(kv_mask)
    q_pos_idx = np.asarray(q_pos_idx)
    kv_pos_idx = np.asarray(kv_pos_idx)
    Wq, Wk, Wv, Wo = (np.asarray(x, np.float32) for x in (Wq, Wk, Wv, Wo))
    bq, bk, bv, bo = (np.asarray(x, np.float32) for x in (bq, bk, bv, bo))

    if "nc" not in _NC_CACHE:
        _NC_CACHE["nc"] = build_nc()
    nc = _NC_CACHE["nc"]

    w_maps = {
        "Wq": np.ascontiguousarray(Wq.reshape(4, P, E)).astype(ml_dtypes.bfloat16),
        "Wk": np.ascontiguousarray(Wk.reshape(4, P, E)).astype(ml_dtypes.bfloat16),
        "Wv": np.ascontiguousarray(Wv.reshape(4, P, E)).astype(ml_dtypes.bfloat16),
        "Wo": np.ascontiguousarray(Wo.reshape(H, Dh, E)).astype(ml_dtypes.bfloat16),
        "bq": np.ascontiguousarray(bq.reshape(4, P).T),
        "bk": np.ascontiguousarray(bk.reshape(4, P).T),
        "bv": np.ascontiguousarray(bv[None, :]).astype(ml_dtypes.bfloat16),
        "bo": np.ascontiguousarray(bo[None, :]).astype(ml_dtypes.bfloat16),
    }

    in_maps = []
    for c in range(NC):
        b, qo = c // 2, (c % 2) * QS
        coskv_t, sinkv_t = _cos_sin_tiles(kv_pos_idx[b])
        cosq_t, sinq_t = _cos_sin_tiles(q_pos_idx[b, qo:qo + QS])
        mneg = np.where(kv_mask[b], 0.0, -1e9).astype(np.float32)
        m = {
            "qT": np.ascontiguousarray(q[b, qo:qo + QS, :].T.reshape(4, P, QS)).astype(ml_dtypes.bfloat16),
            "kvT": np.ascontiguousarray(kv[b].T.reshape(4, P, N)).astype(ml_dtypes.bfloat16),
            "cosq": cosq_t, "sinq": sinq_t,
            "coskv": coskv_t, "sinkv": sinkv_t,
            "maskneg": np.ascontiguousarray(mneg[None, :]).astype(ml_dtypes.bfloat16),
            "maskbias": np.ascontiguousarray(mneg.reshape(16, P).T),
            "qmask": np.ascontiguousarray(
                q_mask[b, qo:qo + QS].astype(np.float32).reshape(4, P).T),
        }
        m.update(w_maps)
        in_maps.append(m)

    res = run_bass_kernel_spmd(nc, in_maps, core_ids=list(range(NC)))

    ctx = np.empty((B, QT, E), np.float32)
    attn = np.empty((B, H, QT, N), np.float32)
    for c in range(NC):
        b, qo = c // 2, (c % 2) * QS
        ctx[b, qo:qo + QS, :] = res.results[c]["ctx_o"]
        attn[b, :, qo:qo + QS, :] = res.results[c]["attn_o"]
    return ctx, attn
